# revision 21
# baseline (speedup 1.0000x reference)
"""GQA attention block (16 q heads / 2 kv heads, RoPE, causal) on 8 TRN2 NeuronCores.

Strategy: tensor-parallel over heads. Each core owns 2 q heads + the matching
kv head (kv heads replicated over 4-core groups), computes its partial o_proj
output over the full sequence, and the host sums the 8 partials. All cores run
the identical graph; only the input *data* differs per core (SPMD-safe).

Dataflow (everything "transposed" so no on-chip transpose of activations is
ever needed):
  - host passes x^T (fp16) once; projections compute Q^T with the weight
    chunk stationary and x^T streaming
  - K^T / V are computed for one sequence *quarter* per core (the 4 cores
    sharing a kv head each take one quarter, selected host-side so the graph
    is identical) and completed via an AllGather over the 4-core group
  - RoPE head-dim is host-permuted so rotate-half partners sit on adjacent
    partitions: the swap is a DVE within-quadrant stream_shuffle
  - scores are computed transposed: S^T[key, q] = K^T_chunk.T @ Q^T
  - softmax without max-subtraction, shifted: P = exp(s*scale - 6) in fp16
    (the shift cancels in the ratio and keeps everything fp16-safe; fp16
    keeps the DVE denominator accumulation in its 2x perf mode)
  - causal masking multiplies the diagonal-band chunks with 0/1 masks
  - denominator = ones-vector matmul over a DVE fp16 accumulation of P^T
  - PV accumulates out^T[d, q] with V (natural layout, via PE transpose)
    stationary and P^T streaming
  - o_proj uses out^T slices as the stationary operand directly; the 1/denom
    scale is applied to out^T on its way out of PSUM; o_proj for window j-1
    is interleaved between the two heads of window j to keep PE fed
"""

import os
import sys

for _p in ("/opt/trn_rl_repo",):
    if os.path.isdir(_p) and _p not in sys.path:
        sys.path.append(_p)

import numpy as np
import ml_dtypes

FP16 = np.float16
BF16 = ml_dtypes.bfloat16

# ---- problem constants (hardcoded per harness contract) ----
S = 4096          # sequence length
H = 2048          # hidden
DH = 128          # head dim
N_CORES = 8
HC = H // 128     # 16 hidden chunks
W = 512           # q-window width
NW = S // W       # 8 windows
SQ = S // 4       # sequence quarter (per-core K/V share)
SCALE = 1.0 / float(np.sqrt(DH))
EXP_SHIFT = -6.0

_CACHE = {}


def _build():
    import concourse.bacc as bacc
    import concourse.mybir as mybir
    import concourse.tile as tile
    from concourse.masks import make_identity

    dt = mybir.dt
    AF = mybir.ActivationFunctionType

    nc = bacc.Bacc("TRN2", target_bir_lowering=False, debug=False,
                   num_devices=N_CORES)

    xt = nc.dram_tensor("xt", [NW, 128, HC * W], dt.bfloat16, kind="ExternalInput")
    wq = nc.dram_tensor("wq", [H, 2 * DH], dt.bfloat16, kind="ExternalInput")
    wk = nc.dram_tensor("wk", [H, DH], dt.bfloat16, kind="ExternalInput")
    wv = nc.dram_tensor("wv", [H, DH], dt.bfloat16, kind="ExternalInput")
    wo = nc.dram_tensor("wo", [2 * DH, H], dt.bfloat16, kind="ExternalInput")
    bqd = nc.dram_tensor("bq", [128, 2], dt.float32, kind="ExternalInput")
    bkvd = nc.dram_tensor("bkv", [128, 2], dt.float32, kind="ExternalInput")
    cosd = nc.dram_tensor("cost", [128, S], dt.bfloat16, kind="ExternalInput")
    sind = nc.dram_tensor("sins", [128, S], dt.bfloat16, kind="ExternalInput")
    mskd = nc.dram_tensor("msk", [128, 4 * W], dt.bfloat16, kind="ExternalInput")
    out = nc.dram_tensor("out", [S, H], dt.float32, kind="ExternalOutput")

    def cview(t):
        # [c*128, n] dram tensor -> [128, c, n] AP (chunk-major in free dim)
        return t.ap().rearrange("(c p) n -> p c n", p=128)

    with tile.TileContext(nc) as tc:
        with (
            tc.tile_pool(name="const", bufs=1) as constp,
            tc.tile_pool(name="xtp", bufs=2) as xtp,
            tc.tile_pool(name="proj", bufs=1) as projp,
            tc.tile_pool(name="ptp", bufs=8) as ptp,
            tc.tile_pool(name="work", bufs=2) as workp,
            tc.tile_pool(name="otsp", bufs=5) as otsp,
            tc.tile_pool(name="obp", bufs=2) as obp,
            tc.tile_pool(name="pp", bufs=2, space="PSUM") as pp,
            tc.tile_pool(name="pqk", bufs=2, space="PSUM") as pqk,
            tc.tile_pool(name="ppv", bufs=2, space="PSUM") as ppv,
        ):
            # ---------- constants into SBUF ----------
            wq_sb = constp.tile([128, HC * 2 * DH], dt.bfloat16, tag="wq")
            wk_sb = constp.tile([128, HC * DH], dt.bfloat16, tag="wk")
            wv_sb = constp.tile([128, HC * DH], dt.bfloat16, tag="wv")
            wo_sb = constp.tile([128, 2 * H], dt.bfloat16, tag="wo")
            bq_sb = constp.tile([128, 2], dt.float32, tag="bq")
            bkv_sb = constp.tile([128, 2], dt.float32, tag="bkv")
            cos_sb = constp.tile([128, S], dt.bfloat16, tag="cos")
            sin_sb = constp.tile([128, S], dt.bfloat16, tag="sin")
            msk_sb = constp.tile([128, 4 * W], dt.bfloat16, tag="msk")
            ones_sb = constp.tile([128, 1], dt.float16, tag="ones")
            ident = constp.tile([128, 128], dt.bfloat16, tag="ident")
            negC = constp.tile([128, 1], dt.float32, tag="negC")

            nc.sync.dma_start(wq_sb[:], cview(wq))
            nc.gpsimd.dma_start(wk_sb[:], cview(wk))
            nc.gpsimd.dma_start(wv_sb[:], cview(wv))
            nc.gpsimd.dma_start(bq_sb[:], bqd[:, :])
            nc.gpsimd.dma_start(bkv_sb[:], bkvd[:, :])
            nc.gpsimd.dma_start(cos_sb[:], cosd[:, :])
            nc.gpsimd.dma_start(sin_sb[:], sind[:, :])
            nc.gpsimd.dma_start(msk_sb[:], mskd[:, :])
            nc.gpsimd.dma_start(wo_sb[:], cview(wo))
            nc.gpsimd.memset(ones_sb[:], 1.0)
            nc.gpsimd.memset(negC[:], EXP_SHIFT)
            make_identity(nc, ident[:])

            qt_sb = projp.tile([128, 2 * S], dt.bfloat16, tag="qt")
            kt_q = [projp.tile([128, SQ], dt.bfloat16, tag=f"ktq{r}",
                               name=f"ktq{r}") for r in range(4)]
            vn_q = [projp.tile([128, SQ], dt.bfloat16, tag=f"vnq{r}",
                               name=f"vnq{r}") for r in range(4)]

            def kt_chunk(k):
                return kt_q[k // 8][:, (k % 8) * 128:(k % 8 + 1) * 128]

            def vn_chunk(k):
                return vn_q[k // 8][:, (k % 8) * 128:(k % 8 + 1) * 128]

            shuffle_mask = [i ^ 1 for i in range(32)]

            def rope_store(ps, bias, dest_slc, cslc, sslc):
                t0 = workp.tile([128, W], dt.bfloat16, tag="rope0")
                nc.vector.tensor_scalar_add(t0[:], ps[:], bias)
                tsw = workp.tile([128, W], dt.bfloat16, tag="ropesw")
                nc.vector.stream_shuffle(tsw[:], t0[:], mask=shuffle_mask)
                t1 = workp.tile([128, W], dt.bfloat16, tag="rope1")
                nc.vector.tensor_mul(t1[:], t0[:], cslc)
                t2 = workp.tile([128, W], dt.bfloat16, tag="rope2")
                nc.vector.tensor_mul(t2[:], tsw[:], sslc)
                nc.vector.tensor_add(dest_slc, t1[:], t2[:])

            # ---------- phase 1: Q^T (2 heads) + K^T + V projections ----------
            for sb in range(NW):
                xb = xtp.tile([128, HC * W], dt.bfloat16, tag="xtb")
                nc.sync.dma_start(xb[:], xt[sb, :, :])
                targets = [
                    ("rope", lambda h: wq_sb[:, h * 256:h * 256 + 128],
                     bq_sb[:, 0:1], qt_sb, 0, cos_sb, sin_sb),
                    ("rope", lambda h: wq_sb[:, h * 256 + 128:h * 256 + 256],
                     bq_sb[:, 1:2], qt_sb, S, cos_sb, sin_sb),
                    ("rope", lambda h: wk_sb[:, h * 128:(h + 1) * 128],
                     bkv_sb[:, 0:1], kt_q[sb // 2], -(sb // 2) * 2 * W,
                     cos_sb, sin_sb),
                    ("vnat", lambda h: wv_sb[:, h * 128:(h + 1) * 128],
                     bkv_sb[:, 1:2], vn_q[sb // 2], 0, None, None),
                ]
                for kind, wslc, bias, dest, doff, ctab, stab in targets:
                    ps = pp.tile([128, W], dt.float32, tag="pp_ps")
                    for h in range(HC):
                        nc.tensor.matmul(
                            ps[:], wslc(h), xb[:, h * W:(h + 1) * W],
                            start=(h == 0), stop=(h == HC - 1))
                    if kind == "rope":
                        rope_store(ps, bias,
                                   dest[:, doff + sb * W: doff + (sb + 1) * W],
                                   ctab[:, sb * W:(sb + 1) * W],
                                   stab[:, sb * W:(sb + 1) * W])
                    else:
                        t0 = workp.tile([128, W], dt.bfloat16, tag="vstage")
                        nc.vector.tensor_scalar_add(t0[:], ps[:], bias)
                        for i in range(W // 128):
                            tp = ppv.tile([128, 128], dt.bfloat16, tag="ppv_ps")
                            nc.tensor.transpose(
                                tp[:], t0[:, i * 128:(i + 1) * 128], ident[:])
                            nc.vector.tensor_copy(
                                dest[:, ((sb % 2) * 4 + i) * 128:
                                     ((sb % 2) * 4 + i + 1) * 128],
                                tp[:])

            # ---------- phase 2: attention, with o_proj(j-1) interleaved ----------
            def attn_head(a, j):
                nkc = 4 * j + 4
                qslc = qt_sb[:, a * S + j * W: a * S + (j + 1) * W]
                ot = ppv.tile([128, W], dt.float32, tag="ppv_ps")
                dacc = workp.tile([128, 2 * W], dt.float16, tag="dacc")
                for g in range(nkc // 2):
                    ps = pqk.tile([128, 2 * W], dt.float32, tag="qk_ps")
                    ptg = ptp.tile([128, 2 * W], dt.bfloat16, tag="pt")
                    for r in range(2):
                        k = 2 * g + r
                        nc.tensor.matmul(
                            ps[:, r * W:(r + 1) * W],
                            kt_chunk(k),
                            qslc, start=True, stop=True)
                    nc.scalar.activation(ptg[:], ps[:], AF.Exp,
                                         scale=SCALE, bias=negC[:])
                    if g >= nkc // 2 - 2:
                        gg = g - (nkc // 2 - 2)   # 0 or 1
                        nc.vector.tensor_mul(
                            ptg[:], ptg[:],
                            msk_sb[:, gg * 2 * W:(gg + 1) * 2 * W])
                    if g == 0:
                        nc.vector.tensor_copy(dacc[:], ptg[:])
                    else:
                        nc.vector.tensor_add(dacc[:], dacc[:], ptg[:])
                    for r in range(2):
                        k = 2 * g + r
                        nc.tensor.matmul(
                            ot[:], vn_chunk(k),
                            ptg[:, r * W:(r + 1) * W],
                            start=(k == 0), stop=(k == nkc - 1))
                dn = ppv.tile([128, W], dt.float32, tag="ppv_ps")
                nc.tensor.matmul(dn[0:1, :], ones_sb[:, 0:1],
                                 dacc[:, 0:W], start=True, stop=False)
                nc.tensor.matmul(dn[0:1, :], ones_sb[:, 0:1],
                                 dacc[:, W:2 * W], start=False, stop=True)
                drc = workp.tile([1, W], dt.float32, tag="drc")
                nc.vector.reciprocal_approx_fast(drc[:], dn[0:1, :])
                drb = workp.tile([128, W], dt.float32, tag="drb")
                nc.gpsimd.partition_broadcast(drb[:], drc[:])
                ots = otsp.tile([128, W], dt.bfloat16, tag="ots")
                nc.vector.tensor_mul(ots[:], ot[:], drb[:])
                return ots

            def oproj(j, ots_heads):
                for qc in range(W // 128):
                    ob = obp.tile([128, H], dt.float32, tag="ob")
                    for n in range(H // W):
                        po = pp.tile([128, W], dt.float32, tag="pp_ps")
                        for a in range(2):
                            nc.tensor.matmul(
                                po[:],
                                ots_heads[a][:, qc * 128:(qc + 1) * 128],
                                wo_sb[:, a * H + n * W: a * H + (n + 1) * W],
                                start=(a == 0), stop=(a == 1))
                        nc.vector.tensor_copy(ob[:, n * W:(n + 1) * W], po[:])
                    nc.sync.dma_start(
                        out[j * W + qc * 128: j * W + (qc + 1) * 128, :], ob[:])

            for j in range(NW):
                o0 = attn_head(0, j)
                o1 = attn_head(1, j)
                oproj(j, (o0, o1))

    nc.compile()
    return nc


def _prep_inputs(x, cos, sin, Wq, bq, Wk, bk, Wv, bv, Wo):
    x = np.asarray(x, dtype=np.float32).reshape(S, H)
    cos = np.asarray(cos, dtype=np.float32).reshape(S, DH)
    sin = np.asarray(sin, dtype=np.float32).reshape(S, DH)

    xtT = x.T.astype(BF16)                       # [H, S]
    # blocked layout: [seq_block, partition, hid_chunk * W] so each block's
    # DMA is one fully-contiguous read
    xtb = np.ascontiguousarray(
        xtT.reshape(HC, 128, NW, W).transpose(2, 1, 0, 3).reshape(NW, 128, HC * W))

    # head-dim permutation: partition 2t <- dim t, partition 2t+1 <- dim t+64
    perm = np.empty(DH, np.int64)
    perm[0::2] = np.arange(64)
    perm[1::2] = np.arange(64) + 64

    cosT = np.ascontiguousarray(cos.T)          # [128, S]
    sinT = np.ascontiguousarray(sin.T)
    cosP = np.ascontiguousarray(cosT[perm]).astype(BF16)
    sinsP = np.empty_like(sinT)
    sinsP[0::2] = -sinT[:64]
    sinsP[1::2] = sinT[:64]
    sinsP = np.ascontiguousarray(sinsP).astype(BF16)

    # causal 0/1 masks for the 4 diagonal-band chunks of each 512-q window
    kk = np.arange(128)[:, None]
    qq = np.arange(W)[None, :]
    msk = np.concatenate(
        [(qq >= kk + 128 * r).astype(np.float32) for r in range(4)],
        axis=1).astype(BF16)

    Wq = np.asarray(Wq, np.float32)
    Wk = np.asarray(Wk, np.float32)
    Wv = np.asarray(Wv, np.float32)
    Wo = np.asarray(Wo, np.float32)
    bq = np.asarray(bq, np.float32)
    bk = np.asarray(bk, np.float32)
    bv = np.asarray(bv, np.float32)

    in_maps = []
    for c in range(N_CORES):
        kv = c // 4
        qtr = c % 4                      # this core's K/V sequence quarter
        # q/k projections get the RoPE head-dim permutation applied to their
        # output columns (and biases); v/o stay in natural order
        wq_c = np.concatenate(
            [Wq[:, (2 * c + a) * DH:(2 * c + a + 1) * DH][:, perm]
             for a in range(2)], axis=1)
        wk_c = Wk[:, kv * DH:(kv + 1) * DH][:, perm]
        wv_c = Wv[:, kv * DH:(kv + 1) * DH]
        wo_c = Wo[2 * c * DH:(2 * c + 2) * DH, :]
        bq_c = np.stack(
            [bq[(2 * c + a) * DH:(2 * c + a + 1) * DH][perm] for a in range(2)],
            axis=1)
        bkv_c = np.stack(
            [bk[kv * DH:(kv + 1) * DH][perm], bv[kv * DH:(kv + 1) * DH]],
            axis=1)
        in_maps.append({
            "xt": xtb,
            "wq": np.ascontiguousarray(wq_c).astype(BF16),
            "wk": np.ascontiguousarray(wk_c).astype(BF16),
            "wv": np.ascontiguousarray(wv_c).astype(BF16),
            "wo": np.ascontiguousarray(wo_c).astype(BF16),
            "bq": np.ascontiguousarray(bq_c).astype(np.float32),
            "bkv": np.ascontiguousarray(bkv_c).astype(np.float32),
            "cost": cosP, "sins": sinsP,
            "msk": msk,
        })
    return in_maps


def _get_nc():
    if "nc" not in _CACHE:
        _CACHE["nc"] = _build()
    return _CACHE["nc"]


def run(trace=False, tmpdir=None, **inputs):
    from concourse.bass_utils import run_bass_kernel_spmd

    nc = _get_nc()
    in_maps = _prep_inputs(**inputs)
    kw = {}
    if trace:
        kw = dict(trace=True, tmpdir=tmpdir)
    res = run_bass_kernel_spmd(nc, in_maps, core_ids=list(range(N_CORES)), **kw)
    acc = np.zeros((S, H), dtype=np.float32)
    for r in res.results:
        acc += r["out"]
    return acc.reshape(1, S, H), res


def kernel(**inputs) -> np.ndarray:
    out, _ = run(**inputs)
    return out


# revision 22
# speedup vs baseline: 1.0016x; 1.0016x over previous
"""GQA attention block (16 q heads / 2 kv heads, RoPE, causal) on 8 TRN2 NeuronCores.

Strategy: tensor-parallel over heads. Each core owns 2 q heads + the matching
kv head (kv heads replicated over 4-core groups), computes its partial o_proj
output over the full sequence, and the host sums the 8 partials. All cores run
the identical graph; only the input *data* differs per core (SPMD-safe).

Dataflow (everything "transposed" so no on-chip transpose of activations is
ever needed):
  - host passes x^T (fp16) once; projections compute Q^T with the weight
    chunk stationary and x^T streaming
  - K^T / V are computed for one sequence *quarter* per core (the 4 cores
    sharing a kv head each take one quarter, selected host-side so the graph
    is identical) and completed via an AllGather over the 4-core group
  - RoPE head-dim is host-permuted so rotate-half partners sit on adjacent
    partitions: the swap is a DVE within-quadrant stream_shuffle
  - scores are computed transposed: S^T[key, q] = K^T_chunk.T @ Q^T
  - softmax without max-subtraction, shifted: P = exp(s*scale - 6) in fp16
    (the shift cancels in the ratio and keeps everything fp16-safe; fp16
    keeps the DVE denominator accumulation in its 2x perf mode)
  - causal masking multiplies the diagonal-band chunks with 0/1 masks
  - denominator = ones-vector matmul over a DVE fp16 accumulation of P^T
  - PV accumulates out^T[d, q] with V (natural layout, via PE transpose)
    stationary and P^T streaming
  - o_proj uses out^T slices as the stationary operand directly; the 1/denom
    scale is applied to out^T on its way out of PSUM; o_proj for window j-1
    is interleaved between the two heads of window j to keep PE fed
"""

import os
import sys

for _p in ("/opt/trn_rl_repo",):
    if os.path.isdir(_p) and _p not in sys.path:
        sys.path.append(_p)

import numpy as np
import ml_dtypes

FP16 = np.float16
BF16 = ml_dtypes.bfloat16

# ---- problem constants (hardcoded per harness contract) ----
S = 4096          # sequence length
H = 2048          # hidden
DH = 128          # head dim
N_CORES = 8
HC = H // 128     # 16 hidden chunks
W = 512           # q-window width
NW = S // W       # 8 windows
SQ = S // 4       # sequence quarter (per-core K/V share)
SCALE = 1.0 / float(np.sqrt(DH))
EXP_SHIFT = -6.0

_CACHE = {}


def _build():
    import concourse.bacc as bacc
    import concourse.mybir as mybir
    import concourse.tile as tile
    from concourse.masks import make_identity

    dt = mybir.dt
    AF = mybir.ActivationFunctionType

    nc = bacc.Bacc("TRN2", target_bir_lowering=False, debug=False,
                   num_devices=N_CORES)

    xt = nc.dram_tensor("xt", [NW, 128, HC * W], dt.bfloat16, kind="ExternalInput")
    wq = nc.dram_tensor("wq", [128, HC * 2 * DH], dt.bfloat16, kind="ExternalInput")
    wk = nc.dram_tensor("wk", [128, HC * DH], dt.bfloat16, kind="ExternalInput")
    wv = nc.dram_tensor("wv", [128, HC * DH], dt.bfloat16, kind="ExternalInput")
    wo = nc.dram_tensor("wo", [128, 2 * H], dt.bfloat16, kind="ExternalInput")
    bqd = nc.dram_tensor("bq", [128, 2], dt.float32, kind="ExternalInput")
    bkvd = nc.dram_tensor("bkv", [128, 2], dt.float32, kind="ExternalInput")
    cosd = nc.dram_tensor("cost", [128, S], dt.bfloat16, kind="ExternalInput")
    sind = nc.dram_tensor("sins", [128, S], dt.bfloat16, kind="ExternalInput")
    mskd = nc.dram_tensor("msk", [128, 4 * W], dt.bfloat16, kind="ExternalInput")
    out = nc.dram_tensor("out", [S, H], dt.float32, kind="ExternalOutput")

    def cview(t):
        # [c*128, n] dram tensor -> [128, c, n] AP (chunk-major in free dim)
        return t.ap().rearrange("(c p) n -> p c n", p=128)

    with tile.TileContext(nc) as tc:
        with (
            tc.tile_pool(name="const", bufs=1) as constp,
            tc.tile_pool(name="xtp", bufs=2) as xtp,
            tc.tile_pool(name="proj", bufs=1) as projp,
            tc.tile_pool(name="ptp", bufs=8) as ptp,
            tc.tile_pool(name="work", bufs=2) as workp,
            tc.tile_pool(name="otsp", bufs=5) as otsp,
            tc.tile_pool(name="obp", bufs=3) as obp,
            tc.tile_pool(name="pp", bufs=2, space="PSUM") as pp,
            tc.tile_pool(name="pqk", bufs=2, space="PSUM") as pqk,
            tc.tile_pool(name="ppv", bufs=2, space="PSUM") as ppv,
        ):
            # ---------- constants into SBUF ----------
            wq_sb = constp.tile([128, HC * 2 * DH], dt.bfloat16, tag="wq")
            wk_sb = constp.tile([128, HC * DH], dt.bfloat16, tag="wk")
            wv_sb = constp.tile([128, HC * DH], dt.bfloat16, tag="wv")
            wo_sb = constp.tile([128, 2 * H], dt.bfloat16, tag="wo")
            bq_sb = constp.tile([128, 2], dt.float32, tag="bq")
            bkv_sb = constp.tile([128, 2], dt.float32, tag="bkv")
            cos_sb = constp.tile([128, S], dt.bfloat16, tag="cos")
            sin_sb = constp.tile([128, S], dt.bfloat16, tag="sin")
            msk_sb = constp.tile([128, 4 * W], dt.bfloat16, tag="msk")
            ones_sb = constp.tile([128, 1], dt.float16, tag="ones")
            ident = constp.tile([128, 128], dt.bfloat16, tag="ident")
            negC = constp.tile([128, 1], dt.float32, tag="negC")

            nc.sync.dma_start(wq_sb[:], wq[:, :])
            nc.gpsimd.dma_start(wk_sb[:], wk[:, :])
            nc.gpsimd.dma_start(wv_sb[:], wv[:, :])
            nc.gpsimd.dma_start(bq_sb[:], bqd[:, :])
            nc.gpsimd.dma_start(bkv_sb[:], bkvd[:, :])
            nc.gpsimd.dma_start(cos_sb[:], cosd[:, :])
            nc.gpsimd.dma_start(sin_sb[:], sind[:, :])
            nc.gpsimd.dma_start(msk_sb[:], mskd[:, :])
            nc.gpsimd.dma_start(wo_sb[:], wo[:, :])
            nc.gpsimd.memset(ones_sb[:], 1.0)
            nc.gpsimd.memset(negC[:], EXP_SHIFT)
            make_identity(nc, ident[:])

            qt_sb = projp.tile([128, 2 * S], dt.bfloat16, tag="qt")
            kt_q = [projp.tile([128, SQ], dt.bfloat16, tag=f"ktq{r}",
                               name=f"ktq{r}") for r in range(4)]
            vn_q = [projp.tile([128, SQ], dt.bfloat16, tag=f"vnq{r}",
                               name=f"vnq{r}") for r in range(4)]

            def kt_chunk(k):
                return kt_q[k // 8][:, (k % 8) * 128:(k % 8 + 1) * 128]

            def vn_chunk(k):
                return vn_q[k // 8][:, (k % 8) * 128:(k % 8 + 1) * 128]

            shuffle_mask = [i ^ 1 for i in range(32)]

            def rope_store(ps, bias, dest_slc, cslc, sslc):
                t0 = workp.tile([128, W], dt.bfloat16, tag="rope0")
                nc.vector.tensor_scalar_add(t0[:], ps[:], bias)
                tsw = workp.tile([128, W], dt.bfloat16, tag="ropesw")
                nc.vector.stream_shuffle(tsw[:], t0[:], mask=shuffle_mask)
                t1 = workp.tile([128, W], dt.bfloat16, tag="rope1")
                nc.vector.tensor_mul(t1[:], t0[:], cslc)
                t2 = workp.tile([128, W], dt.bfloat16, tag="rope2")
                nc.vector.tensor_mul(t2[:], tsw[:], sslc)
                nc.vector.tensor_add(dest_slc, t1[:], t2[:])

            # ---------- phase 1: Q^T (2 heads) + K^T + V projections ----------
            for sb in range(NW):
                xb = xtp.tile([128, HC * W], dt.bfloat16, tag="xtb")
                nc.sync.dma_start(xb[:], xt[sb, :, :])
                targets = [
                    ("rope", lambda h: wq_sb[:, h * 256:h * 256 + 128],
                     bq_sb[:, 0:1], qt_sb, 0, cos_sb, sin_sb),
                    ("rope", lambda h: wq_sb[:, h * 256 + 128:h * 256 + 256],
                     bq_sb[:, 1:2], qt_sb, S, cos_sb, sin_sb),
                    ("rope", lambda h: wk_sb[:, h * 128:(h + 1) * 128],
                     bkv_sb[:, 0:1], kt_q[sb // 2], -(sb // 2) * 2 * W,
                     cos_sb, sin_sb),
                    ("vnat", lambda h: wv_sb[:, h * 128:(h + 1) * 128],
                     bkv_sb[:, 1:2], vn_q[sb // 2], 0, None, None),
                ]
                for kind, wslc, bias, dest, doff, ctab, stab in targets:
                    ps = pp.tile([128, W], dt.float32, tag="pp_ps")
                    for h in range(HC):
                        nc.tensor.matmul(
                            ps[:], wslc(h), xb[:, h * W:(h + 1) * W],
                            start=(h == 0), stop=(h == HC - 1))
                    if kind == "rope":
                        rope_store(ps, bias,
                                   dest[:, doff + sb * W: doff + (sb + 1) * W],
                                   ctab[:, sb * W:(sb + 1) * W],
                                   stab[:, sb * W:(sb + 1) * W])
                    else:
                        t0 = workp.tile([128, W], dt.bfloat16, tag="vstage")
                        nc.vector.tensor_scalar_add(t0[:], ps[:], bias)
                        for i in range(W // 128):
                            tp = ppv.tile([128, 128], dt.bfloat16, tag="ppv_ps")
                            nc.tensor.transpose(
                                tp[:], t0[:, i * 128:(i + 1) * 128], ident[:])
                            nc.vector.tensor_copy(
                                dest[:, ((sb % 2) * 4 + i) * 128:
                                     ((sb % 2) * 4 + i + 1) * 128],
                                tp[:])

            # ---------- phase 2: attention, with o_proj(j-1) interleaved ----------
            def attn_head(a, j):
                nkc = 4 * j + 4
                qslc = qt_sb[:, a * S + j * W: a * S + (j + 1) * W]
                ot = ppv.tile([128, W], dt.float32, tag="ppv_ps")
                dacc = workp.tile([128, 2 * W], dt.float16, tag="dacc")
                for g in range(nkc // 2):
                    ps = pqk.tile([128, 2 * W], dt.float32, tag="qk_ps")
                    ptg = ptp.tile([128, 2 * W], dt.bfloat16, tag="pt")
                    for r in range(2):
                        k = 2 * g + r
                        nc.tensor.matmul(
                            ps[:, r * W:(r + 1) * W],
                            kt_chunk(k),
                            qslc, start=True, stop=True)
                    nc.scalar.activation(ptg[:], ps[:], AF.Exp,
                                         scale=SCALE, bias=negC[:])
                    if g >= nkc // 2 - 2:
                        gg = g - (nkc // 2 - 2)   # 0 or 1
                        nc.vector.tensor_mul(
                            ptg[:], ptg[:],
                            msk_sb[:, gg * 2 * W:(gg + 1) * 2 * W])
                    if g == 0:
                        nc.vector.tensor_copy(dacc[:], ptg[:])
                    else:
                        nc.vector.tensor_add(dacc[:], dacc[:], ptg[:])
                    for r in range(2):
                        k = 2 * g + r
                        nc.tensor.matmul(
                            ot[:], vn_chunk(k),
                            ptg[:, r * W:(r + 1) * W],
                            start=(k == 0), stop=(k == nkc - 1))
                dn = ppv.tile([128, W], dt.float32, tag="ppv_ps")
                nc.tensor.matmul(dn[0:1, :], ones_sb[:, 0:1],
                                 dacc[:, 0:W], start=True, stop=False)
                nc.tensor.matmul(dn[0:1, :], ones_sb[:, 0:1],
                                 dacc[:, W:2 * W], start=False, stop=True)
                drc = workp.tile([1, W], dt.float32, tag="drc")
                nc.vector.reciprocal_approx_fast(drc[:], dn[0:1, :])
                drb = workp.tile([128, W], dt.float32, tag="drb")
                nc.gpsimd.partition_broadcast(drb[:], drc[:])
                ots = otsp.tile([128, W], dt.bfloat16, tag="ots")
                nc.vector.tensor_mul(ots[:], ot[:], drb[:])
                return ots

            def oproj(j, ots_heads):
                for qc in range(W // 128):
                    ob = obp.tile([128, H], dt.float32, tag="ob")
                    for n in range(H // W):
                        po = pp.tile([128, W], dt.float32, tag="pp_ps")
                        for a in range(2):
                            nc.tensor.matmul(
                                po[:],
                                ots_heads[a][:, qc * 128:(qc + 1) * 128],
                                wo_sb[:, a * H + n * W: a * H + (n + 1) * W],
                                start=(a == 0), stop=(a == 1))
                        nc.vector.tensor_copy(ob[:, n * W:(n + 1) * W], po[:])
                    nc.sync.dma_start(
                        out[j * W + qc * 128: j * W + (qc + 1) * 128, :], ob[:])

            for j in range(NW):
                o0 = attn_head(0, j)
                o1 = attn_head(1, j)
                oproj(j, (o0, o1))

    nc.compile()
    return nc


def _prep_inputs(x, cos, sin, Wq, bq, Wk, bk, Wv, bv, Wo):
    x = np.asarray(x, dtype=np.float32).reshape(S, H)
    cos = np.asarray(cos, dtype=np.float32).reshape(S, DH)
    sin = np.asarray(sin, dtype=np.float32).reshape(S, DH)

    xtT = x.T.astype(BF16)                       # [H, S]
    # blocked layout: [seq_block, partition, hid_chunk * W] so each block's
    # DMA is one fully-contiguous read
    xtb = np.ascontiguousarray(
        xtT.reshape(HC, 128, NW, W).transpose(2, 1, 0, 3).reshape(NW, 128, HC * W))

    # head-dim permutation: partition 2t <- dim t, partition 2t+1 <- dim t+64
    perm = np.empty(DH, np.int64)
    perm[0::2] = np.arange(64)
    perm[1::2] = np.arange(64) + 64

    cosT = np.ascontiguousarray(cos.T)          # [128, S]
    sinT = np.ascontiguousarray(sin.T)
    cosP = np.ascontiguousarray(cosT[perm]).astype(BF16)
    sinsP = np.empty_like(sinT)
    sinsP[0::2] = -sinT[:64]
    sinsP[1::2] = sinT[:64]
    sinsP = np.ascontiguousarray(sinsP).astype(BF16)

    # causal 0/1 masks for the 4 diagonal-band chunks of each 512-q window
    kk = np.arange(128)[:, None]
    qq = np.arange(W)[None, :]
    msk = np.concatenate(
        [(qq >= kk + 128 * r).astype(np.float32) for r in range(4)],
        axis=1).astype(BF16)

    Wq = np.asarray(Wq, np.float32)
    Wk = np.asarray(Wk, np.float32)
    Wv = np.asarray(Wv, np.float32)
    Wo = np.asarray(Wo, np.float32)
    bq = np.asarray(bq, np.float32)
    bk = np.asarray(bk, np.float32)
    bv = np.asarray(bv, np.float32)

    in_maps = []
    for c in range(N_CORES):
        kv = c // 4
        qtr = c % 4                      # this core's K/V sequence quarter
        # q/k projections get the RoPE head-dim permutation applied to their
        # output columns (and biases); v/o stay in natural order
        wq_c = np.concatenate(
            [Wq[:, (2 * c + a) * DH:(2 * c + a + 1) * DH][:, perm]
             for a in range(2)], axis=1)
        wk_c = Wk[:, kv * DH:(kv + 1) * DH][:, perm]
        wv_c = Wv[:, kv * DH:(kv + 1) * DH]
        wo_c = Wo[2 * c * DH:(2 * c + 2) * DH, :]
        bq_c = np.stack(
            [bq[(2 * c + a) * DH:(2 * c + a + 1) * DH][perm] for a in range(2)],
            axis=1)
        bkv_c = np.stack(
            [bk[kv * DH:(kv + 1) * DH][perm], bv[kv * DH:(kv + 1) * DH]],
            axis=1)
        def wrearr(w):
            c = w.shape[0] // 128
            return np.ascontiguousarray(
                w.reshape(c, 128, -1).transpose(1, 0, 2).reshape(128, -1))

        in_maps.append({
            "xt": xtb,
            "wq": wrearr(wq_c).astype(BF16),
            "wk": wrearr(wk_c).astype(BF16),
            "wv": wrearr(wv_c).astype(BF16),
            "wo": wrearr(wo_c).astype(BF16),
            "bq": np.ascontiguousarray(bq_c).astype(np.float32),
            "bkv": np.ascontiguousarray(bkv_c).astype(np.float32),
            "cost": cosP, "sins": sinsP,
            "msk": msk,
        })
    return in_maps


def _get_nc():
    if "nc" not in _CACHE:
        _CACHE["nc"] = _build()
    return _CACHE["nc"]


def run(trace=False, tmpdir=None, **inputs):
    from concourse.bass_utils import run_bass_kernel_spmd

    nc = _get_nc()
    in_maps = _prep_inputs(**inputs)
    kw = {}
    if trace:
        kw = dict(trace=True, tmpdir=tmpdir)
    res = run_bass_kernel_spmd(nc, in_maps, core_ids=list(range(N_CORES)), **kw)
    acc = np.zeros((S, H), dtype=np.float32)
    for r in res.results:
        acc += r["out"]
    return acc.reshape(1, S, H), res


def kernel(**inputs) -> np.ndarray:
    out, _ = run(**inputs)
    return out
